# revision 1
# baseline (speedup 1.0000x reference)
"""Multi-head attention (B=2, S=2048, D=1024, H=16, causal, unscaled scores)
on 8 Trainium2 NeuronCores.

Sharding: 2 batches x 4 head-groups (4 heads each). Core c handles batch
c//4, heads 4*(c%4) .. 4*(c%4)+3. Each core computes its group's QKV
projections, causal attention, and a partial output projection
(row-slice of wo); the host sums the 4 partials per batch (the
all-reduce) and adds the bias terms.

Device layout avoids all on-chip transposes:
  - host passes q/k/v transposed ([D, S]) so projections produce
    QHT/KHT = (x@w).T with head-dim on partitions (score-ready)
  - VH is produced in natural [S, D_head] orientation with an extra
    ones column, so the attnV matmul also accumulates the softmax
    denominator (row 64 of U^T)
  - normalization is deferred: U^T is copied out raw (freeing its PSUM
    bank), then per head-pair two accumulating K=1 selector-row matmuls
    broadcast both reciprocal rows into one [128,512] bank and a single
    full-width multiply rescales ct in place; the bias terms bv/bo are
    folded in exactly on the host (C = U/colsum + 1*bv since softmax
    rows sum to 1).
All matmuls run as float32r (bf16-pair fp32: ~1e-4 rel err, 4x the
throughput of plain fp32).
"""

import numpy as np

D = 1024
S = 2048
NH = 16
DH = 64
B = 2
G = 4            # head-groups = cores per batch
HG = NH // G     # 4 heads per group
GD = HG * DH     # 256 columns per group
KT = D // 128    # 8 k-tiles
MS = S // 512    # 4 m-slices
JT = S // 128    # 16 j-tiles
IST = S // 512   # 4 i-slices

_cached = None

_SEL = np.zeros((2, 128), np.float32)
_SEL[0, 0:64] = 1.0
_SEL[1, 64:128] = 1.0


def _build():
    from concourse import bacc
    import concourse.mybir as mybir
    import concourse.tile as tile

    f32 = mybir.dt.float32
    f32r = mybir.dt.float32r
    Act = mybir.ActivationFunctionType
    Alu = mybir.AluOpType

    nc = bacc.Bacc(None, target_bir_lowering=False)
    xq = nc.dram_tensor("xq", [D, S], f32r, kind="ExternalInput")
    xk = nc.dram_tensor("xk", [D, S], f32r, kind="ExternalInput")
    xv = nc.dram_tensor("xv", [D, S], f32r, kind="ExternalInput")
    wqg = nc.dram_tensor("wqg", [D, GD], f32r, kind="ExternalInput")
    wkg = nc.dram_tensor("wkg", [D, GD], f32r, kind="ExternalInput")
    wvg = nc.dram_tensor("wvg", [D, GD], f32r, kind="ExternalInput")
    wog = nc.dram_tensor("wog", [GD, D], f32r, kind="ExternalInput")
    bqg = nc.dram_tensor("bqg", [2, 128, 1], f32, kind="ExternalInput")
    bkg = nc.dram_tensor("bkg", [2, 128, 1], f32, kind="ExternalInput")
    selg = nc.dram_tensor("selg", [2, 128], f32r, kind="ExternalInput")
    outp = nc.dram_tensor("outp", [S, D], f32, kind="ExternalOutput")

    with tile.TileContext(nc) as tc:
        with (
            tc.tile_pool(name="wpool", bufs=1) as wpool,
            tc.tile_pool(name="xres", bufs=2) as xres,
            tc.tile_pool(name="xstr", bufs=8) as xstr,
            tc.tile_pool(name="big", bufs=1) as big,
            tc.tile_pool(name="ppool", bufs=8) as ppool,
            tc.tile_pool(name="small", bufs=6) as small,
            tc.tile_pool(name="osb", bufs=4) as osb,
            tc.tile_pool(name="ps", bufs=2, space="PSUM") as ps,
            tc.tile_pool(name="po", bufs=2, space="PSUM") as po,
            tc.tile_pool(name="psU", bufs=2, space="PSUM") as psU,
        ):
            # ---- resident weights / constants ----
            wq_t = wpool.tile([128, KT, GD], f32r, tag="wq")
            wk_t = wpool.tile([128, KT, GD], f32r, tag="wk")
            wv_t = wpool.tile([128, KT, GD], f32r, tag="wv")
            wo_t = wpool.tile([128, 2, D], f32r, tag="wo")
            bq_t = wpool.tile([128, 2, 1], f32, tag="bq")
            bk_t = wpool.tile([128, 2, 1], f32, tag="bk")
            sel0 = wpool.tile([1, 128], f32r, tag="sel0")
            sel1 = wpool.tile([1, 128], f32r, tag="sel1")
            wql = xq_like_w(wqg)
            nc.sync.dma_start(out=wq_t[:, 0:2, :], in_=wql[:, 0:2, :])
            nc.sync.dma_start(out=wq_t[:, 2:KT, :], in_=wql[:, 2:KT, :])
            nc.sync.dma_start(out=bq_t, in_=bqg[:].rearrange("t p o -> p t o"))
            nc.sync.dma_start(out=bk_t, in_=bkg[:].rearrange("t p o -> p t o"))
            nc.sync.dma_start(out=sel0, in_=selg[0:1, :])
            nc.sync.dma_start(out=sel1, in_=selg[1:2, :])

            qht = big.tile([128, 2, S], f32r, tag="qht")
            kht = big.tile([128, 2, S], f32r, tag="kht")
            vh = big.tile([128, JT, HG, DH + 1], f32r, tag="vh")
            ct = big.tile([128, 2, S], f32r, tag="ct")
            vh_ones_stage = wpool.tile([128, JT, HG, 1], f32, tag="vh_ones_st")
            nc.vector.memset(vh_ones_stage, 1.0)
            nc.scalar.activation(
                out=vh[:, :, :, DH : DH + 1], in_=vh_ones_stage, func=Act.Copy
            )

            # ---- projections, interleaved per m-slice so attention(IS=0)'s
            # inputs (QHT/KHT m=0, VH j=0..3) are produced first ----
            for m in range(MS):
                ms = slice(m * 512, (m + 1) * 512)
                # stream x slices for this m
                xq_ts, xk_ts = [], []
                for kk in range(KT):
                    xt = xstr.tile([128, 512], f32r, tag="xt", name="xqt")
                    nc.sync.dma_start(out=xt, in_=xq[kk * 128 : (kk + 1) * 128, ms])
                    xq_ts.append(xt)
                if m == 0:
                    nc.sync.dma_start(out=wk_t, in_=xq_like_w(wkg))
                for kk in range(KT):
                    xt = xstr.tile([128, 512], f32r, tag="xt", name="xkt")
                    nc.sync.dma_start(out=xt, in_=xk[kk * 128 : (kk + 1) * 128, ms])
                    xk_ts.append(xt)
                if m == 0:
                    nc.sync.dma_start(out=wv_t, in_=xq_like_w(wvg))
                xv_t = xres.tile([128, KT, 512], f32r, tag="xv")
                for kk in range(KT):
                    nc.sync.dma_start(
                        out=xv_t[:, kk, :], in_=xv[kk * 128 : (kk + 1) * 128, ms]
                    )
                if m == 0:
                    nc.sync.dma_start(
                        out=wo_t, in_=wog[:].rearrange("(t p) n -> p t n", p=128)
                    )
                # QHT / KHT for this m (bias added on DVE during PSUM->SBUF)
                for xts, w_t, b_t, dst in (
                    (xq_ts, wq_t, bq_t, qht),
                    (xk_ts, wk_t, bk_t, kht),
                ):
                    for n in range(2):
                        psum = po.tile([128, 512], f32, tag="po")
                        for kk in range(KT):
                            nc.tensor.matmul(
                                psum,
                                w_t[:, kk, n * 128 : (n + 1) * 128],
                                xts[kk],
                                start=(kk == 0),
                                stop=(kk == KT - 1),
                            )
                        nc.vector.tensor_scalar_add(dst[:, n, ms], psum, b_t[:, n, :])
                # VH for this m (j-tiles 4m..4m+3), single strided copy per tile
                for jj in range(4):
                    j = m * 4 + jj
                    psum = po.tile([128, GD], f32, tag="po")
                    for kk in range(KT):
                        nc.tensor.matmul(
                            psum,
                            xv_t[:, kk, jj * 128 : (jj + 1) * 128],
                            wv_t[:, kk, :],
                            start=(kk == 0),
                            stop=(kk == KT - 1),
                        )
                    nc.vector.tensor_copy(
                        vh[:, j, :, 0:DH],
                        psum[:].rearrange("p (h d) -> p h d", h=HG),
                    )

            # ---- attention + output projection ----
            # heads processed in pairs occupying partitions 0-63 / 64-127 so
            # their K=64 score matmuls run concurrently in the PE array.
            # U^T is copied to ct raw (releases U banks immediately);
            # normalization (reciprocal + broadcast + mul) is deferred to the
            # i-slice boundary, and the output projection runs one i-slice
            # behind attention so PE never idles on the normalize chain.
            def emit_attention(IS, between_pairs=None):
                i0 = IS * 512
                n_j = (IS + 1) * 4
                recips = {}
                for hp in range(HG // 2):
                    if hp > 0 and between_pairs is not None:
                        between_pairs()
                    nt = hp  # pair hp covers heads 2*hp, 2*hp+1 = kht tile hp
                    u_psums = [
                        psU.tile([128, 512], f32, tag="u", name=f"u{e}")
                        for e in range(2)
                    ]
                    # work units: fused pairs of full j-tiles below the
                    # diagonal band, then per-tile units with the fully-masked
                    # column prefix trimmed (tile J computes cols [J*128-i0, 512))
                    n_full = n_j - 4  # tiles strictly below the diagonal band
                    units = []
                    for Jg in range(n_full // 2):
                        units.append(("full", Jg))
                    for J in range(n_full, n_j):
                        units.append(("diag", J))
                    pts = {}
                    s_psums = {}

                    def emit_scores(u):
                        kind, idx = u
                        if kind == "full":
                            for e in range(2):
                                lo = 64 * e
                                s_psum = ps.tile([128, 2, 512], f32, tag="ps")
                                for half in range(2):
                                    J = 2 * idx + half
                                    nc.tensor.matmul(
                                        s_psum[:, half, :],
                                        kht[lo : lo + DH, nt, J * 128 : (J + 1) * 128],
                                        qht[lo : lo + DH, nt, i0 : i0 + 512],
                                        start=True,
                                        stop=True,
                                    )
                                s_psums[(e, u)] = s_psum
                        else:
                            J = idx
                            r = J * 128 - i0
                            s_psum = ps.tile([128, 2, 512], f32, tag="ps", name="sd")
                            for e in range(2):
                                lo = 64 * e
                                nc.tensor.matmul(
                                    s_psum[:, e, 0 : 512 - r],
                                    kht[lo : lo + DH, nt, J * 128 : (J + 1) * 128],
                                    qht[lo : lo + DH, nt, i0 + r : i0 + 512],
                                    start=True,
                                    stop=True,
                                )
                            s_psums[(0, u)] = s_psum

                    def emit_exp_mask(u):
                        kind, idx = u
                        if kind == "full":
                            for e in range(2):
                                pt = ppool.tile([128, 2, 512], f32r, tag="pt")
                                nc.scalar.activation(
                                    out=pt, in_=s_psums[(e, u)], func=Act.Exp
                                )
                                pts[(e, u)] = pt
                        else:
                            r = idx * 128 - i0
                            w = 512 - r
                            pt = ppool.tile([128, 2, 512], f32r, tag="pt", name="ptd")
                            nc.scalar.activation(
                                out=pt[:, :, 0:w],
                                in_=s_psums[(0, u)][:, :, 0:w],
                                func=Act.Exp,
                            )
                            # keep col x >= partition p (relative to r), same
                            # predicate for both head-halves (step-0 dim)
                            nc.gpsimd.affine_select(
                                out=pt[:, :, 0:w],
                                in_=pt[:, :, 0:w],
                                compare_op=Alu.is_ge,
                                fill=0.0,
                                base=0,
                                pattern=[[0, 2], [1, w]],
                                channel_multiplier=-1,
                            )
                            pts[(0, u)] = pt

                    def emit_attnv(u):
                        kind, idx = u
                        for e in range(2):
                            if kind == "full":
                                for half in range(2):
                                    J = 2 * idx + half
                                    nc.tensor.matmul(
                                        u_psums[e][0 : DH + 1, :],
                                        vh[:, J, 2 * hp + e, :],
                                        pts[(e, u)][:, half, :],
                                        start=(J == 0),
                                        stop=False,
                                    )
                            else:
                                J = idx
                                r = J * 128 - i0
                                nc.tensor.matmul(
                                    u_psums[e][0 : DH + 1, r:512],
                                    vh[:, J, 2 * hp + e, :],
                                    pts[(0, u)][:, e, 0 : 512 - r],
                                    start=(J == 0),
                                    stop=(J == n_j - 1),
                                )

                    # software pipeline: scores run 1 unit ahead of attnV
                    emit_scores(units[0])
                    emit_exp_mask(units[0])
                    for ui in range(1, len(units)):
                        emit_scores(units[ui])
                        emit_exp_mask(units[ui])
                        emit_attnv(units[ui - 1])
                    emit_attnv(units[-1])

                    # release U banks fast: copy raw U^T out, keep 1/colsum
                    for e in range(2):
                        lo = 64 * e
                        recip = small.tile([1, 512], f32r, tag="recip", name=f"rc{e}")
                        with nc.allow_low_precision(reason="fp32r is fp32-width"):
                            nc.vector.reciprocal(recip, u_psums[e][DH : DH + 1, :])
                        nc.vector.tensor_copy(
                            ct[lo : lo + DH, nt, i0 : i0 + 512], u_psums[e][0:DH, :]
                        )
                        recips[(hp, e)] = recip
                    if hp > 0:
                        emit_normalize_pair(IS, hp - 1, recips)
                emit_normalize_pair(IS, HG // 2 - 1, recips)
                return recips

            def emit_normalize_pair(IS, hp, recips):
                i0 = IS * 512
                bc_psum = ps.tile([128, 512], f32, tag="ps", name="bcp")
                for e, sel in ((0, sel0), (1, sel1)):
                    nc.tensor.matmul(
                        bc_psum, sel, recips[(hp, e)], start=(e == 0), stop=(e == 1)
                    )
                # in0 is PSUM, so the matching-SB-base-partition rule doesn't
                # bind; multiply straight into ct in place, full width
                nc.vector.tensor_mul(
                    ct[:, hp, i0 : i0 + 512],
                    bc_psum,
                    ct[:, hp, i0 : i0 + 512],
                )

            def emit_outproj(IS):
                i0 = IS * 512
                for it in range(4):
                    r0 = i0 + it * 128
                    out_sb = osb.tile([128, D], f32, tag="out")
                    for nn in range(2):
                        o_psum = po.tile([128, 512], f32, tag="po")
                        for t in range(2):
                            nc.tensor.matmul(
                                o_psum,
                                ct[:, t, r0 : r0 + 128],
                                wo_t[:, t, nn * 512 : (nn + 1) * 512],
                                start=(t == 0),
                                stop=(t == 1),
                            )
                        nc.vector.tensor_copy(out_sb[:, nn * 512 : (nn + 1) * 512], o_psum)
                    nc.sync.dma_start(out=outp[r0 : r0 + 128, :], in_=out_sb)

            prev = None
            for IS in range(IST):
                recips = emit_attention(IS)
                if prev is not None:
                    emit_outproj(prev)
                prev = IS
            emit_outproj(prev)

    nc.compile()
    return nc


def xq_like_w(w):
    return w[:].rearrange("(kt p) n -> p kt n", p=128)


def _get_nc():
    global _cached
    if _cached is None:
        _cached = _build()
    return _cached


def _in_maps(q, k, v, wq, bq, wk, bk, wv, bv, wo, bo):
    maps = []
    for c in range(8):
        b, g = c // G, c % G
        cs = slice(g * GD, (g + 1) * GD)
        maps.append(
            {
                "xq": np.ascontiguousarray(q[b].T).astype(np.float32, copy=False),
                "xk": np.ascontiguousarray(k[b].T).astype(np.float32, copy=False),
                "xv": np.ascontiguousarray(v[b].T).astype(np.float32, copy=False),
                "wqg": np.ascontiguousarray(wq[:, cs]),
                "wkg": np.ascontiguousarray(wk[:, cs]),
                "wvg": np.ascontiguousarray(wv[:, cs]),
                "wog": np.ascontiguousarray(wo[cs, :]),
                "bqg": np.ascontiguousarray(bq[cs]).reshape(2, 128, 1),
                "bkg": np.ascontiguousarray(bk[cs]).reshape(2, 128, 1),
                "selg": _SEL,
            }
        )
    return maps


def run(inputs, trace=False, trace_kwargs=None):
    from concourse.bass_utils import run_bass_kernel_spmd

    nc = _get_nc()
    maps = _in_maps(**inputs)
    res = run_bass_kernel_spmd(
        nc, maps, list(range(8)), trace=trace, **(trace_kwargs or {})
    )
    q = inputs["q"]
    out = np.zeros((B, S, D), np.float32)
    for c in range(8):
        out[c // G] += res.results[c]["outp"]
    # exact bias fold: C = U/colsum + 1 (x) bv  =>  out += bv @ wo + bo
    out += inputs["bv"].astype(np.float32) @ inputs["wo"].astype(np.float32)
    out += inputs["bo"].astype(np.float32)
    return out.astype(np.float32), res


def kernel(**inputs) -> np.ndarray:
    out, _ = run(inputs)
    return out



# revision 6
# speedup vs baseline: 1.0389x; 1.0389x over previous
"""Multi-head attention (B=2, S=2048, D=1024, H=16, causal, unscaled scores)
on 8 Trainium2 NeuronCores.

Sharding: 2 batches x 4 head-groups (4 heads each). Core c handles batch
c//4, heads 4*(c%4) .. 4*(c%4)+3. Each core computes its group's QKV
projections, causal attention, and a partial output projection
(row-slice of wo); the host sums the 4 partials per batch (the
all-reduce) and adds the bias terms.

Precision: Q/K path (xq, xk, wq, wk, qht, kht, scores) stays float32r
(bf16-pair fp32, ~1e-4) so the exp() arguments are accurate; the V path
(xv, wv, vh, exp-probabilities, attention output, wo, final output) is
bf16 — measured end-to-end rel err ~4e-3 against fp32, well inside the
2e-2 gate, and it halves HBM traffic + DVE cost on that side.

Schedule (v2, PE-density-first):
  - 16 warmup outer-product matmuls at t~0 keep the PE HAM activity
    window busy so the real stream starts at 2.4 GHz.
  - x/w loads are one strided DMA per (tensor, m-slice); xq/xk go on
    the sync HWDGE ring, xv/weights/outputs on the scalar ring so big
    Q/K streams never head-of-line-block V loads or output stores.
  - projection work is emitted in small chunks INSIDE the attention
    unit loop (attention i-slice IS overlaps projections m=IS+1), so
    the PE never stalls on the x DMA stream.
  - output projection runs one i-slice behind attention; softmax
    normalization is deferred: denominator rows (accumulated by the
    ones-column of VH during attnV) are copied to SBUF right away
    (freeing the U PSUM banks), reciprocals run batched [2,512] per
    head-pair on DVE off the critical path, and a K=2 selector matmul
    broadcasts both reciprocal rows into a [128,512] bank for one
    full-width in-place multiply of ct.
  - bias terms bv/bo are folded in exactly on the host
    (C = U/colsum + 1*bv since softmax rows sum to 1).
"""

import numpy as np

D = 1024
S = 2048
NH = 16
DH = 64
B = 2
G = 4            # head-groups = cores per batch
HG = NH // G     # 4 heads per group
GD = HG * DH     # 256 columns per group
KT = D // 128    # 8 k-tiles
MS = S // 512    # 4 m-slices
JT = S // 128    # 16 j-tiles
IST = S // 512   # 4 i-slices

_cached = None

_SEL = np.zeros((2, 128), np.float32)
_SEL[0, 0:64] = 1.0
_SEL[1, 64:128] = 1.0


def _build():
    from concourse import bacc
    import concourse.mybir as mybir
    import concourse.tile as tile

    f32 = mybir.dt.float32
    f32r = mybir.dt.float32r
    bf16 = mybir.dt.bfloat16
    Act = mybir.ActivationFunctionType
    Alu = mybir.AluOpType

    nc = bacc.Bacc(None, target_bir_lowering=False)
    xq = nc.dram_tensor("xq", [D, S], f32r, kind="ExternalInput")
    xk = nc.dram_tensor("xk", [D, S], f32r, kind="ExternalInput")
    xv = nc.dram_tensor("xv", [D, S], bf16, kind="ExternalInput")
    wqg = nc.dram_tensor("wqg", [D, GD], f32r, kind="ExternalInput")
    wkg = nc.dram_tensor("wkg", [D, GD], f32r, kind="ExternalInput")
    wvg = nc.dram_tensor("wvg", [D, GD], bf16, kind="ExternalInput")
    wog = nc.dram_tensor("wog", [GD, D], bf16, kind="ExternalInput")
    bqg = nc.dram_tensor("bqg", [2, 128, 1], f32, kind="ExternalInput")
    bkg = nc.dram_tensor("bkg", [2, 128, 1], f32, kind="ExternalInput")
    selg = nc.dram_tensor("selg", [2, 128], f32r, kind="ExternalInput")
    outp = nc.dram_tensor("outp", [S, D], bf16, kind="ExternalOutput")

    with tile.TileContext(nc) as tc:
        with (
            tc.tile_pool(name="wpool", bufs=1) as wpool,
            tc.tile_pool(name="xqk", bufs=2) as xqk,
            tc.tile_pool(name="xvs", bufs=2) as xvs,
            tc.tile_pool(name="big", bufs=1) as big,
            tc.tile_pool(name="ppool", bufs=8) as ppool,
            tc.tile_pool(name="small", bufs=4) as small,
            tc.tile_pool(name="osb", bufs=4) as osb,
            tc.tile_pool(name="ps", bufs=2, space="PSUM") as ps,
            tc.tile_pool(name="po", bufs=2, space="PSUM") as po,
            tc.tile_pool(name="psU", bufs=2, space="PSUM") as psU,
        ):
            # ---- resident weights / constants ----
            wq_t = wpool.tile([128, KT, GD], f32r, tag="wq")
            wk_t = wpool.tile([128, KT, GD], f32r, tag="wk")
            wv_t = wpool.tile([128, KT, GD], bf16, tag="wv")
            wo_t = wpool.tile([128, 2, D], bf16, tag="wo")
            bq_t = wpool.tile([128, 2, 1], f32, tag="bq")
            bk_t = wpool.tile([128, 2, 1], f32, tag="bk")
            sel0 = wpool.tile([1, 128], f32r, tag="sel0")
            sel1 = wpool.tile([1, 128], f32r, tag="sel1")
            warm_sink = wpool.tile([1, 16], f32, tag="wsink")

            # selector rows first (tiny) so warmup matmuls start ~t=0
            nc.sync.dma_start(out=sel0, in_=selg[0:1, :])
            nc.sync.dma_start(out=sel1, in_=selg[1:2, :])

            # ---- PE warmup: ~3.5us of junk outer products so the HAM
            # un-throttles before the first projection matmul ----
            wpsum = po.tile([128, 128], f32, tag="po", name="warm")
            for i in range(16):
                nc.tensor.matmul(
                    wpsum,
                    sel0,
                    sel0,
                    start=(i == 0),
                    stop=(i == 15),
                )
            nc.vector.tensor_copy(warm_sink, wpsum[0:1, 0:16])

            # ---- input streams ----
            # sync ring: wq, xq(m0), wk, xk(m0), then xq/xk m1..3
            # scalar ring: bq, bk, wv, xv(m0), wo, xv m1..3 (+ outputs later)
            nc.sync.dma_start(out=wq_t[:, 0:2, :], in_=xq_like_w(wqg)[:, 0:2, :])
            nc.scalar.dma_start(out=bq_t, in_=bqg[:].rearrange("t p o -> p t o"))
            nc.scalar.dma_start(out=bk_t, in_=bkg[:].rearrange("t p o -> p t o"))

            xq_ts, xk_ts, xv_ts = [], [], []
            xq_r = xq[:].rearrange("(kt p) s -> p kt s", p=128)
            xk_r = xk[:].rearrange("(kt p) s -> p kt s", p=128)
            xv_r = xv[:].rearrange("(kt p) s -> p kt s", p=128)

            def load_m(m):
                ms = slice(m * 512, (m + 1) * 512)
                xqt = xqk.tile([128, KT, 512], f32r, tag="xq", name="xqt")
                xkt = xqk.tile([128, KT, 512], f32r, tag="xk", name="xkt")
                xvt = xvs.tile([128, KT, 512], bf16, tag="xv", name="xvt")
                if m == 0:
                    # split m0 so the first Q matmuls can start sooner
                    nc.sync.dma_start(out=xqt[:, 0:4, :], in_=xq_r[:, 0:4, ms])
                    nc.sync.dma_start(out=xqt[:, 4:KT, :], in_=xq_r[:, 4:KT, ms])
                else:
                    nc.sync.dma_start(out=xqt, in_=xq_r[:, :, ms])
                if m == 0:
                    nc.sync.dma_start(out=wq_t[:, 2:KT, :], in_=xq_like_w(wqg)[:, 2:KT, :])
                    nc.sync.dma_start(out=wk_t, in_=xq_like_w(wkg))
                nc.sync.dma_start(out=xkt, in_=xk_r[:, :, ms])
                if m == 0:
                    nc.scalar.dma_start(out=wv_t, in_=xq_like_w(wvg))
                nc.scalar.dma_start(out=xvt, in_=xv_r[:, :, ms])
                if m == 0:
                    nc.scalar.dma_start(
                        out=wo_t, in_=wog[:].rearrange("(t p) n -> p t n", p=128)
                    )
                xq_ts.append(xqt)
                xk_ts.append(xkt)
                xv_ts.append(xvt)

            for m in range(MS):
                load_m(m)

            # ---- persistent activations ----
            qht = big.tile([128, 2, S], f32r, tag="qht")
            kht = big.tile([128, 2, S], f32r, tag="kht")
            vh = big.tile([128, JT, HG, DH + 1], bf16, tag="vh")
            ct = big.tile([128, 2, S], bf16, tag="ct")
            vh_ones_stage = wpool.tile([128, JT, HG, 1], f32, tag="vh_ones_st")
            nc.vector.memset(vh_ones_stage, 1.0)
            nc.scalar.activation(
                out=vh[:, :, :, DH : DH + 1], in_=vh_ones_stage, func=Act.Copy
            )

            # ---- projection chunk emitters (consumed as fillers) ----
            def proj_chunks(m):
                """Yield small emission callables for m-slice projections."""
                ms = slice(m * 512, (m + 1) * 512)

                def qk_chunk(xts, w_t, b_t, dst, n):
                    def emit():
                        psum = po.tile([128, 512], f32, tag="po")
                        for kk in range(KT):
                            nc.tensor.matmul(
                                psum,
                                w_t[:, kk, n * 128 : (n + 1) * 128],
                                xts[:, kk, :],
                                start=(kk == 0),
                                stop=(kk == KT - 1),
                            )
                        nc.vector.tensor_scalar_add(dst[:, n, ms], psum, b_t[:, n, :])

                    return emit

                def v_chunk(jj):
                    def emit():
                        j = m * 4 + jj
                        psum = po.tile([128, GD], f32, tag="po", name="vps")
                        for kk in range(KT):
                            nc.tensor.matmul(
                                psum,
                                xv_ts[m][:, kk, jj * 128 : (jj + 1) * 128],
                                wv_t[:, kk, :],
                                start=(kk == 0),
                                stop=(kk == KT - 1),
                            )
                        nc.vector.tensor_copy(
                            vh[:, j, :, 0:DH],
                            psum[:].rearrange("p (h d) -> p h d", h=HG),
                        )

                    return emit

                for n in range(2):
                    yield qk_chunk(xq_ts[m], wq_t, bq_t, qht, n)
                for n in range(2):
                    yield qk_chunk(xk_ts[m], wk_t, bk_t, kht, n)
                for jj in range(4):
                    yield v_chunk(jj)

            # ---- attention ----
            recips = {}

            def emit_attention_pair(IS, hp, filler=None):
                """Scores+exp+attnV pipeline for head-pair hp of i-slice IS.
                filler() is called between units to interleave other PE work."""
                i0 = IS * 512
                n_j = (IS + 1) * 4
                nt = hp
                u_psums = [
                    psU.tile([128, 512], f32, tag="u", name=f"u{e}")
                    for e in range(2)
                ]
                n_full = n_j - 4
                units = []
                for Jg in range(n_full // 2):
                    units.append(("full", Jg))
                for J in range(n_full, n_j):
                    units.append(("diag", J))
                pts = {}
                s_psums = {}

                def emit_scores(u):
                    kind, idx = u
                    if kind == "full":
                        for e in range(2):
                            lo = 64 * e
                            s_psum = ps.tile([128, 2, 512], f32, tag="ps")
                            for half in range(2):
                                J = 2 * idx + half
                                nc.tensor.matmul(
                                    s_psum[:, half, :],
                                    kht[lo : lo + DH, nt, J * 128 : (J + 1) * 128],
                                    qht[lo : lo + DH, nt, i0 : i0 + 512],
                                    start=True,
                                    stop=True,
                                )
                            s_psums[(e, u)] = s_psum
                    else:
                        J = idx
                        r = J * 128 - i0
                        s_psum = ps.tile([128, 2, 512], f32, tag="ps", name="sd")
                        for e in range(2):
                            lo = 64 * e
                            nc.tensor.matmul(
                                s_psum[:, e, 0 : 512 - r],
                                kht[lo : lo + DH, nt, J * 128 : (J + 1) * 128],
                                qht[lo : lo + DH, nt, i0 + r : i0 + 512],
                                start=True,
                                stop=True,
                            )
                        s_psums[(0, u)] = s_psum

                def emit_exp_mask(u):
                    kind, idx = u
                    if kind == "full":
                        for e in range(2):
                            pt = ppool.tile([128, 2, 512], bf16, tag="pt")
                            nc.scalar.activation(
                                out=pt, in_=s_psums[(e, u)], func=Act.Exp
                            )
                            pts[(e, u)] = pt
                    else:
                        r = idx * 128 - i0
                        w = 512 - r
                        pt = ppool.tile([128, 2, 512], bf16, tag="pt", name="ptd")
                        nc.scalar.activation(
                            out=pt[:, :, 0:w],
                            in_=s_psums[(0, u)][:, :, 0:w],
                            func=Act.Exp,
                        )
                        nc.gpsimd.affine_select(
                            out=pt[:, :, 0:w],
                            in_=pt[:, :, 0:w],
                            compare_op=Alu.is_ge,
                            fill=0.0,
                            base=0,
                            pattern=[[0, 2], [1, w]],
                            channel_multiplier=-1,
                        )
                        pts[(0, u)] = pt

                def emit_attnv(u):
                    kind, idx = u
                    for e in range(2):
                        if kind == "full":
                            for half in range(2):
                                J = 2 * idx + half
                                nc.tensor.matmul(
                                    u_psums[e][0 : DH + 1, :],
                                    vh[:, J, 2 * hp + e, :],
                                    pts[(e, u)][:, half, :],
                                    start=(J == 0),
                                    stop=False,
                                )
                        else:
                            J = idx
                            r = J * 128 - i0
                            nc.tensor.matmul(
                                u_psums[e][0 : DH + 1, r:512],
                                vh[:, J, 2 * hp + e, :],
                                pts[(0, u)][:, e, 0 : 512 - r],
                                start=(J == 0),
                                stop=(J == n_j - 1),
                            )

                emit_scores(units[0])
                emit_exp_mask(units[0])
                for ui in range(1, len(units)):
                    emit_scores(units[ui])
                    emit_exp_mask(units[ui])
                    emit_attnv(units[ui - 1])
                    if filler is not None:
                        filler()
                emit_attnv(units[-1])

                # evacuate U banks: ct rows out, fast-approx reciprocal of the
                # denominator row straight from PSUM (~5x cheaper than the
                # iterative DVE reciprocal; denominators are strictly positive
                # finite so the approx edge cases can't occur), then a trivial
                # f32 -> f32r convert-copy for the selector matmul.
                for e in range(2):
                    lo = 64 * e
                    nc.vector.tensor_copy(
                        ct[lo : lo + DH, nt, i0 : i0 + 512], u_psums[e][0:DH, :]
                    )
                    rc = small.tile([1, 512], f32r, tag="rc", name=f"rc{e}")
                    with nc.allow_low_precision(reason="fp32r is fp32-width"):
                        nc.vector.reciprocal(rc, u_psums[e][DH : DH + 1, :])
                    recips[(IS, hp, e)] = rc

            def emit_normalize(IS):
                i0 = IS * 512
                for hp in range(HG // 2):
                    bc_psum = po.tile([128, 512], f32, tag="po", name="bcp")
                    for e, sel in ((0, sel0), (1, sel1)):
                        nc.tensor.matmul(
                            bc_psum,
                            sel,
                            recips[(IS, hp, e)],
                            start=(e == 0),
                            stop=(e == 1),
                        )
                    nc.vector.tensor_mul(
                        ct[:, hp, i0 : i0 + 512],
                        bc_psum,
                        ct[:, hp, i0 : i0 + 512],
                    )

            def emit_outproj(IS):
                i0 = IS * 512
                for it in range(4):
                    r0 = i0 + it * 128
                    out_sb = osb.tile([128, D], bf16, tag="out")
                    for nn in range(2):
                        o_psum = po.tile([128, 512], f32, tag="po")
                        for t in range(2):
                            nc.tensor.matmul(
                                o_psum,
                                ct[:, t, r0 : r0 + 128],
                                wo_t[:, t, nn * 512 : (nn + 1) * 512],
                                start=(t == 0),
                                stop=(t == 1),
                            )
                        nc.vector.tensor_copy(out_sb[:, nn * 512 : (nn + 1) * 512], o_psum)
                    nc.scalar.dma_start(out=outp[r0 : r0 + 128, :], in_=out_sb)

            # ---- main schedule ----
            # proj(m0) up front; then per i-slice: attention pair0 (with
            # proj(m=IS+1) chunks as fillers), [normalize+outproj of IS-1],
            # attention pair1 (more fillers), leftover chunks.
            for emit in proj_chunks(0):
                emit()

            for IS in range(IST):
                pending = list(proj_chunks(IS + 1)) if IS + 1 < MS else []
                pending.reverse()  # pop() from the front of the original order

                def filler():
                    if pending:
                        pending.pop()()

                emit_attention_pair(IS, 0, filler=filler)
                if IS > 0:
                    emit_normalize(IS - 1)
                    emit_outproj(IS - 1)
                emit_attention_pair(IS, 1, filler=filler)
                while pending:
                    pending.pop()()
            emit_normalize(IST - 1)
            emit_outproj(IST - 1)

    nc.compile()
    return nc


def xq_like_w(w):
    return w[:].rearrange("(kt p) n -> p kt n", p=128)


def _get_nc():
    global _cached
    if _cached is None:
        _cached = _build()
    return _cached


def _in_maps(q, k, v, wq, bq, wk, bk, wv, bv, wo, bo):
    import ml_dtypes

    bf = ml_dtypes.bfloat16
    maps = []
    for c in range(8):
        b, g = c // G, c % G
        cs = slice(g * GD, (g + 1) * GD)
        maps.append(
            {
                "xq": np.ascontiguousarray(q[b].T).astype(np.float32, copy=False),
                "xk": np.ascontiguousarray(k[b].T).astype(np.float32, copy=False),
                "xv": np.ascontiguousarray(v[b].T).astype(bf),
                "wqg": np.ascontiguousarray(wq[:, cs]),
                "wkg": np.ascontiguousarray(wk[:, cs]),
                "wvg": np.ascontiguousarray(wv[:, cs]).astype(bf),
                "wog": np.ascontiguousarray(wo[cs, :]).astype(bf),
                "bqg": np.ascontiguousarray(bq[cs]).reshape(2, 128, 1),
                "bkg": np.ascontiguousarray(bk[cs]).reshape(2, 128, 1),
                "selg": _SEL,
            }
        )
    return maps


def run(inputs, trace=False, trace_kwargs=None):
    from concourse.bass_utils import run_bass_kernel_spmd

    nc = _get_nc()
    maps = _in_maps(**inputs)
    res = run_bass_kernel_spmd(
        nc, maps, list(range(8)), trace=trace, **(trace_kwargs or {})
    )
    out = np.zeros((B, S, D), np.float32)
    for c in range(8):
        out[c // G] += res.results[c]["outp"].astype(np.float32)
    # exact bias fold: C = U/colsum + 1 (x) bv  =>  out += bv @ wo + bo
    out += inputs["bv"].astype(np.float32) @ inputs["wo"].astype(np.float32)
    out += inputs["bo"].astype(np.float32)
    return out.astype(np.float32), res


def kernel(**inputs) -> np.ndarray:
    out, _ = run(inputs)
    return out


# revision 7
# speedup vs baseline: 1.6009x; 1.5410x over previous
"""Multi-head attention (B=2, S=2048, D=1024, H=16, causal, unscaled scores)
on 8 Trainium2 NeuronCores.

Sharding: 2 batches x 4 head-groups (4 heads each). Core c handles batch
c//4, heads 4*(c%4) .. 4*(c%4)+3. Each core computes its group's QKV
projections, causal attention, and a partial output projection
(row-slice of wo); the host sums the 4 partials per batch (the
all-reduce) and adds the bias terms.

Precision: Q/K path (xq, xk, wq, wk, qht, kht, scores) stays float32r
(bf16-pair fp32, ~1e-4) so the exp() arguments are accurate; the V path
(xv, wv, vh, exp-probabilities, attention output, wo, final output) is
bf16 — measured end-to-end rel err ~4e-3 against fp32, well inside the
2e-2 gate, and it halves HBM traffic + DVE cost on that side.

Schedule (v2, PE-density-first):
  - 16 warmup outer-product matmuls at t~0 keep the PE HAM activity
    window busy so the real stream starts at 2.4 GHz.
  - x/w loads are one strided DMA per (tensor, m-slice); xq/xk go on
    the sync HWDGE ring, xv/weights/outputs on the scalar ring so big
    Q/K streams never head-of-line-block V loads or output stores.
  - projection work is emitted in small chunks INSIDE the attention
    unit loop (attention i-slice IS overlaps projections m=IS+1), so
    the PE never stalls on the x DMA stream.
  - output projection runs one i-slice behind attention; softmax
    normalization is deferred: denominator rows (accumulated by the
    ones-column of VH during attnV) are copied to SBUF right away
    (freeing the U PSUM banks), reciprocals run batched [2,512] per
    head-pair on DVE off the critical path, and a K=2 selector matmul
    broadcasts both reciprocal rows into a [128,512] bank for one
    full-width in-place multiply of ct.
  - bias terms bv/bo are folded in exactly on the host
    (C = U/colsum + 1*bv since softmax rows sum to 1).
"""

import numpy as np

D = 1024
S = 2048
NH = 16
DH = 64
B = 2
G = 4            # head-groups = cores per batch
HG = NH // G     # 4 heads per group
GD = HG * DH     # 256 columns per group
KT = D // 128    # 8 k-tiles
MS = S // 512    # 4 m-slices
JT = S // 128    # 16 j-tiles
IST = S // 512   # 4 i-slices

_cached = None

_SEL = np.zeros((2, 128), np.float32)
_SEL[0, 0:64] = 1.0
_SEL[1, 64:128] = 1.0


def _build():
    from concourse import bacc
    import concourse.mybir as mybir
    import concourse.tile as tile

    f32 = mybir.dt.float32
    f32r = mybir.dt.float32r
    f16 = mybir.dt.float16
    bf16 = mybir.dt.bfloat16
    Act = mybir.ActivationFunctionType
    Alu = mybir.AluOpType

    nc = bacc.Bacc(None, target_bir_lowering=False)
    xq = nc.dram_tensor("xq", [D, S], f16, kind="ExternalInput")
    xk = nc.dram_tensor("xk", [D, S], f16, kind="ExternalInput")
    xv = nc.dram_tensor("xv", [D, S], bf16, kind="ExternalInput")
    wqg = nc.dram_tensor("wqg", [D, GD], f16, kind="ExternalInput")
    wkg = nc.dram_tensor("wkg", [D, GD], f16, kind="ExternalInput")
    wvg = nc.dram_tensor("wvg", [D, GD], bf16, kind="ExternalInput")
    wog = nc.dram_tensor("wog", [GD, D], bf16, kind="ExternalInput")
    bqg = nc.dram_tensor("bqg", [2, 128, 1], f32, kind="ExternalInput")
    bkg = nc.dram_tensor("bkg", [2, 128, 1], f32, kind="ExternalInput")
    selg = nc.dram_tensor("selg", [2, 128], f32r, kind="ExternalInput")
    outp = nc.dram_tensor("outp", [S, D], bf16, kind="ExternalOutput")

    with tile.TileContext(nc) as tc:
        with (
            tc.tile_pool(name="wpool", bufs=1) as wpool,
            tc.tile_pool(name="xqk", bufs=2) as xqk,
            tc.tile_pool(name="xvs", bufs=2) as xvs,
            tc.tile_pool(name="big", bufs=1) as big,
            tc.tile_pool(name="ppool", bufs=8) as ppool,
            tc.tile_pool(name="small", bufs=4) as small,
            tc.tile_pool(name="osb", bufs=4) as osb,
            tc.tile_pool(name="ps", bufs=2, space="PSUM") as ps,
            tc.tile_pool(name="po", bufs=2, space="PSUM") as po,
            tc.tile_pool(name="psU", bufs=2, space="PSUM") as psU,
        ):
            # ---- resident weights / constants ----
            wq_t = wpool.tile([128, KT, GD], f16, tag="wq")
            wk_t = wpool.tile([128, KT, GD], f16, tag="wk")
            wv_t = wpool.tile([128, KT, GD], bf16, tag="wv")
            wo_t = wpool.tile([128, 2, D], bf16, tag="wo")
            bq_t = wpool.tile([128, 2, 1], f32, tag="bq")
            bk_t = wpool.tile([128, 2, 1], f32, tag="bk")
            sel0 = wpool.tile([1, 128], f32r, tag="sel0")
            sel1 = wpool.tile([1, 128], f32r, tag="sel1")
            warm_sink = wpool.tile([1, 16], f32, tag="wsink")

            # selector rows first (tiny) so warmup matmuls start ~t=0
            nc.sync.dma_start(out=sel0, in_=selg[0:1, :])
            nc.sync.dma_start(out=sel1, in_=selg[1:2, :])

            # ---- PE warmup: ~3.5us of junk outer products so the HAM
            # un-throttles before the first projection matmul ----
            wpsum = po.tile([128, 128], f32, tag="po", name="warm")
            for i in range(32):
                nc.tensor.matmul(
                    wpsum,
                    sel0,
                    sel0,
                    start=(i == 0),
                    stop=(i == 31),
                )
            nc.vector.tensor_copy(warm_sink, wpsum[0:1, 0:16])

            # ---- input streams ----
            # sync ring: wq, xq(m0), wk, xk(m0), then xq/xk m1..3
            # scalar ring: bq, bk, wv, xv(m0), wo, xv m1..3 (+ outputs later)
            nc.sync.dma_start(out=wq_t[:, 0:2, :], in_=xq_like_w(wqg)[:, 0:2, :])
            nc.scalar.dma_start(out=bq_t, in_=bqg[:].rearrange("t p o -> p t o"))
            nc.scalar.dma_start(out=bk_t, in_=bkg[:].rearrange("t p o -> p t o"))

            xq_ts, xk_ts, xv_ts = [], [], []
            xq_r = xq[:].rearrange("(kt p) s -> p kt s", p=128)
            xk_r = xk[:].rearrange("(kt p) s -> p kt s", p=128)
            xv_r = xv[:].rearrange("(kt p) s -> p kt s", p=128)

            def load_m(m):
                ms = slice(m * 512, (m + 1) * 512)
                xqt = xqk.tile([128, KT, 512], f16, tag="xq", name="xqt")
                xkt = xqk.tile([128, KT, 512], f16, tag="xk", name="xkt")
                xvt = xvs.tile([128, KT, 512], bf16, tag="xv", name="xvt")
                if m == 0:
                    # split m0 so the first Q matmuls can start sooner
                    nc.sync.dma_start(out=xqt[:, 0:4, :], in_=xq_r[:, 0:4, ms])
                    nc.sync.dma_start(out=xqt[:, 4:KT, :], in_=xq_r[:, 4:KT, ms])
                else:
                    nc.sync.dma_start(out=xqt, in_=xq_r[:, :, ms])
                if m == 0:
                    nc.sync.dma_start(out=wq_t[:, 2:KT, :], in_=xq_like_w(wqg)[:, 2:KT, :])
                    nc.sync.dma_start(out=wk_t, in_=xq_like_w(wkg))
                nc.sync.dma_start(out=xkt, in_=xk_r[:, :, ms])
                if m == 0:
                    nc.sync.dma_start(out=wv_t, in_=xq_like_w(wvg))
                nc.sync.dma_start(out=xvt, in_=xv_r[:, :, ms])
                if m == 0:
                    nc.sync.dma_start(
                        out=wo_t, in_=wog[:].rearrange("(t p) n -> p t n", p=128)
                    )
                xq_ts.append(xqt)
                xk_ts.append(xkt)
                xv_ts.append(xvt)

            for m in range(MS):
                load_m(m)

            # ---- persistent activations ----
            qht = big.tile([128, 2, S], f16, tag="qht")
            kht = big.tile([128, 2, S], f16, tag="kht")
            vh = big.tile([128, JT, HG, DH + 1], bf16, tag="vh")
            ct = big.tile([128, 2, S], bf16, tag="ct")
            vh_ones_stage = wpool.tile([128, JT, HG, 1], f32, tag="vh_ones_st")
            nc.vector.memset(vh_ones_stage, 1.0)
            nc.scalar.activation(
                out=vh[:, :, :, DH : DH + 1], in_=vh_ones_stage, func=Act.Copy
            )

            # ---- projection chunk emitters (consumed as fillers) ----
            def proj_chunks(m):
                """Yield small emission callables for m-slice projections."""
                ms = slice(m * 512, (m + 1) * 512)

                def qk_chunk(xts, w_t, b_t, dst, n):
                    def emit():
                        psum = po.tile([128, 512], f32, tag="po")
                        for kk in range(KT):
                            nc.tensor.matmul(
                                psum,
                                w_t[:, kk, n * 128 : (n + 1) * 128],
                                xts[:, kk, :],
                                start=(kk == 0),
                                stop=(kk == KT - 1),
                            )
                        nc.vector.tensor_scalar_add(dst[:, n, ms], psum, b_t[:, n, :])

                    return emit

                def v_chunk(jj):
                    def emit():
                        j = m * 4 + jj
                        psum = po.tile([128, GD], f32, tag="po", name="vps")
                        for kk in range(KT):
                            nc.tensor.matmul(
                                psum,
                                xv_ts[m][:, kk, jj * 128 : (jj + 1) * 128],
                                wv_t[:, kk, :],
                                start=(kk == 0),
                                stop=(kk == KT - 1),
                            )
                        nc.vector.tensor_copy(
                            vh[:, j, :, 0:DH],
                            psum[:].rearrange("p (h d) -> p h d", h=HG),
                        )

                    return emit

                for n in range(2):
                    yield qk_chunk(xq_ts[m], wq_t, bq_t, qht, n)
                for n in range(2):
                    yield qk_chunk(xk_ts[m], wk_t, bk_t, kht, n)
                for jj in range(4):
                    yield v_chunk(jj)

            # ---- attention ----
            recips = {}

            def emit_attention_pair(IS, hp, filler=None):
                """Scores+exp+attnV pipeline for head-pair hp of i-slice IS.
                filler() is called between units to interleave other PE work."""
                i0 = IS * 512
                n_j = (IS + 1) * 4
                nt = hp
                u_psums = [
                    psU.tile([128, 512], f32, tag="u", name=f"u{e}")
                    for e in range(2)
                ]
                n_full = n_j - 4
                units = []
                for Jg in range(n_full // 2):
                    units.append(("full", Jg))
                for J in range(n_full, n_j):
                    units.append(("diag", J))
                pts = {}
                s_psums = {}

                def emit_scores(u):
                    kind, idx = u
                    if kind == "full":
                        for e in range(2):
                            lo = 64 * e
                            s_psum = ps.tile([128, 2, 512], f32, tag="ps")
                            for half in range(2):
                                J = 2 * idx + half
                                nc.tensor.matmul(
                                    s_psum[:, half, :],
                                    kht[lo : lo + DH, nt, J * 128 : (J + 1) * 128],
                                    qht[lo : lo + DH, nt, i0 : i0 + 512],
                                    start=True,
                                    stop=True,
                                )
                            s_psums[(e, u)] = s_psum
                    else:
                        J = idx
                        r = J * 128 - i0
                        s_psum = ps.tile([128, 2, 512], f32, tag="ps", name="sd")
                        for e in range(2):
                            lo = 64 * e
                            nc.tensor.matmul(
                                s_psum[:, e, 0 : 512 - r],
                                kht[lo : lo + DH, nt, J * 128 : (J + 1) * 128],
                                qht[lo : lo + DH, nt, i0 + r : i0 + 512],
                                start=True,
                                stop=True,
                            )
                        s_psums[(0, u)] = s_psum

                def emit_exp_mask(u):
                    kind, idx = u
                    if kind == "full":
                        for e in range(2):
                            pt = ppool.tile([128, 2, 512], bf16, tag="pt")
                            nc.scalar.activation(
                                out=pt, in_=s_psums[(e, u)], func=Act.Exp
                            )
                            pts[(e, u)] = pt
                    else:
                        r = idx * 128 - i0
                        w = 512 - r
                        pt = ppool.tile([128, 2, 512], bf16, tag="pt", name="ptd")
                        nc.scalar.activation(
                            out=pt[:, :, 0:w],
                            in_=s_psums[(0, u)][:, :, 0:w],
                            func=Act.Exp,
                        )
                        nc.gpsimd.affine_select(
                            out=pt[:, :, 0:w],
                            in_=pt[:, :, 0:w],
                            compare_op=Alu.is_ge,
                            fill=0.0,
                            base=0,
                            pattern=[[0, 2], [1, w]],
                            channel_multiplier=-1,
                        )
                        pts[(0, u)] = pt

                def emit_attnv(u):
                    kind, idx = u
                    for e in range(2):
                        if kind == "full":
                            for half in range(2):
                                J = 2 * idx + half
                                nc.tensor.matmul(
                                    u_psums[e][0 : DH + 1, :],
                                    vh[:, J, 2 * hp + e, :],
                                    pts[(e, u)][:, half, :],
                                    start=(J == 0),
                                    stop=False,
                                )
                        else:
                            J = idx
                            r = J * 128 - i0
                            nc.tensor.matmul(
                                u_psums[e][0 : DH + 1, r:512],
                                vh[:, J, 2 * hp + e, :],
                                pts[(0, u)][:, e, 0 : 512 - r],
                                start=(J == 0),
                                stop=(J == n_j - 1),
                            )

                emit_scores(units[0])
                emit_exp_mask(units[0])
                for ui in range(1, len(units)):
                    emit_scores(units[ui])
                    emit_exp_mask(units[ui])
                    emit_attnv(units[ui - 1])
                    if filler is not None:
                        filler()
                emit_attnv(units[-1])

                # evacuate U banks: ct rows out, fast-approx reciprocal of the
                # denominator row straight from PSUM (~5x cheaper than the
                # iterative DVE reciprocal; denominators are strictly positive
                # finite so the approx edge cases can't occur), then a trivial
                # f32 -> f32r convert-copy for the selector matmul.
                for e in range(2):
                    lo = 64 * e
                    nc.vector.tensor_copy(
                        ct[lo : lo + DH, nt, i0 : i0 + 512], u_psums[e][0:DH, :]
                    )
                    # NOTE: reciprocal_approx_fast's custom ucode ignores the
                    # partition offset on PSUM reads (HW-probed), so stage the
                    # denominator row to SBUF partition 0 first.  The staging
                    # copy also releases the U PSUM bank quickly.
                    rden = small.tile([1, 512], f32, tag="rden", name=f"rd{e}")
                    rcf = small.tile([1, 512], f32, tag="rcf", name=f"rcf{e}")
                    rc = small.tile([1, 512], f32r, tag="rc", name=f"rc{e}")
                    nc.vector.tensor_copy(rden, u_psums[e][DH : DH + 1, :])
                    nc.vector.reciprocal_approx_fast(out=rcf, in_=rden)
                    nc.vector.tensor_copy(rc, rcf)
                    recips[(IS, hp, e)] = rc

            def emit_normalize(IS):
                i0 = IS * 512
                for hp in range(HG // 2):
                    bc_psum = po.tile([128, 512], f32, tag="po", name="bcp")
                    for e, sel in ((0, sel0), (1, sel1)):
                        nc.tensor.matmul(
                            bc_psum,
                            sel,
                            recips[(IS, hp, e)],
                            start=(e == 0),
                            stop=(e == 1),
                        )
                    nc.vector.tensor_mul(
                        ct[:, hp, i0 : i0 + 512],
                        bc_psum,
                        ct[:, hp, i0 : i0 + 512],
                    )

            def emit_outproj(IS):
                i0 = IS * 512
                for it in range(4):
                    r0 = i0 + it * 128
                    out_sb = osb.tile([128, D], bf16, tag="out")
                    for nn in range(2):
                        o_psum = po.tile([128, 512], f32, tag="po")
                        for t in range(2):
                            nc.tensor.matmul(
                                o_psum,
                                ct[:, t, r0 : r0 + 128],
                                wo_t[:, t, nn * 512 : (nn + 1) * 512],
                                start=(t == 0),
                                stop=(t == 1),
                            )
                        nc.vector.tensor_copy(out_sb[:, nn * 512 : (nn + 1) * 512], o_psum)
                    nc.gpsimd.dma_start(out=outp[r0 : r0 + 128, :], in_=out_sb)

            # ---- main schedule ----
            # proj(m0) up front; then per i-slice: attention pair0 (with
            # proj(m=IS+1) chunks as fillers), [normalize+outproj of IS-1],
            # attention pair1 (more fillers), leftover chunks.
            for emit in proj_chunks(0):
                emit()

            for IS in range(IST):
                pending = list(proj_chunks(IS + 1)) if IS + 1 < MS else []
                pending.reverse()  # pop() from the front of the original order

                def filler():
                    if pending:
                        pending.pop()()

                emit_attention_pair(IS, 0, filler=filler)
                if IS > 0:
                    emit_normalize(IS - 1)
                    emit_outproj(IS - 1)
                emit_attention_pair(IS, 1, filler=filler)
                while pending:
                    pending.pop()()
            emit_normalize(IST - 1)
            emit_outproj(IST - 1)

    nc.compile()
    return nc


def xq_like_w(w):
    return w[:].rearrange("(kt p) n -> p kt n", p=128)


def _get_nc():
    global _cached
    if _cached is None:
        _cached = _build()
    return _cached


def _in_maps(q, k, v, wq, bq, wk, bk, wv, bv, wo, bo):
    import ml_dtypes

    bf = ml_dtypes.bfloat16
    maps = []
    for c in range(8):
        b, g = c // G, c % G
        cs = slice(g * GD, (g + 1) * GD)
        maps.append(
            {
                "xq": np.ascontiguousarray(q[b].T).astype(np.float16),
                "xk": np.ascontiguousarray(k[b].T).astype(np.float16),
                "xv": np.ascontiguousarray(v[b].T).astype(bf),
                "wqg": np.ascontiguousarray(wq[:, cs]).astype(np.float16),
                "wkg": np.ascontiguousarray(wk[:, cs]).astype(np.float16),
                "wvg": np.ascontiguousarray(wv[:, cs]).astype(bf),
                "wog": np.ascontiguousarray(wo[cs, :]).astype(bf),
                "bqg": np.ascontiguousarray(bq[cs]).reshape(2, 128, 1),
                "bkg": np.ascontiguousarray(bk[cs]).reshape(2, 128, 1),
                "selg": _SEL,
            }
        )
    return maps


def run(inputs, trace=False, trace_kwargs=None):
    from concourse.bass_utils import run_bass_kernel_spmd

    nc = _get_nc()
    maps = _in_maps(**inputs)
    res = run_bass_kernel_spmd(
        nc, maps, list(range(8)), trace=trace, **(trace_kwargs or {})
    )
    out = np.zeros((B, S, D), np.float32)
    for c in range(8):
        out[c // G] += res.results[c]["outp"].astype(np.float32)
    # exact bias fold: C = U/colsum + 1 (x) bv  =>  out += bv @ wo + bo
    out += inputs["bv"].astype(np.float32) @ inputs["wo"].astype(np.float32)
    out += inputs["bo"].astype(np.float32)
    return out.astype(np.float32), res


def kernel(**inputs) -> np.ndarray:
    out, _ = run(inputs)
    return out
